# revision 26
# baseline (speedup 1.0000x reference)
"""Trainium2 Bass kernel for 16-head MultiHeadAttention (B=2, S=2048, D=1024).

Sharding: 8 cores = 2 (batch) x 4 (head groups of 4 heads).  Each core
computes, for its batch b and head group g:
  Q_g = x_q @ Wq[:, g] ; K_g, V_g likewise
  ctx_g = softmax(Q_g K_g^T / sqrt(64)) V_g            (4 heads)
  out_partial = ctx_g @ Wo[g, :]                        [2048, 1024]
Host sums the 4 partials per batch and adds bo.

v3 layout/schedule notes:
  - inputs/weights are pre-cast to fp16 AND pre-tiled on the host so every
    DMA moves fully contiguous 8KB-per-partition lines at peak HBM rate;
    output partials are fp16
  - activations are fed transposed (features on partitions) so every matmul
    contracts over the partition dim without any on-device transposes
  - scores are computed transposed (s^T[keys, queries]) so the exp'd
    probabilities feed the ctx matmul directly; softmax skips
    max-subtraction (scores ~ N(0,1)); denominators come from a ones
    column appended to V
  - PSUM: tag A [128,2,512]x2 (double-buffered scores + phase-1
    projections), tag C [128,2,512]x1 (ctx accum, both heads), tag O
    [128,1024]x1 (out-proj + mid-attention filler projections) = 8 banks
  - one global software pipeline over all 128 (qt, hp, kc) steps: ctx
    matmuls lag the score matmuls by LAG steps, and out-proj / Q-proj /
    V-proj work is injected as "filler" units between steps so the
    in-order PE stream never blocks the exp cadence on ScalarE (the
    bottleneck engine)
  - softmax: ctx+denominator rows are copied out of PSUM first (frees the
    C accumulator after ~1.3us), then 1/den via reciprocal_approx_fast
    (~51 ULP) + gpsimd partition_broadcast + multiply, off the PE
    critical path.  Custom-DVE ops ignore the input base partition, so
    the denominator row is relocated to p0 by a builtin copy first.
"""

import os
import sys

sys.path.insert(0, "/opt/trn_rl_repo")

import numpy as np

import concourse.bass as bass
import concourse.tile as tile
from concourse import bacc, mybir
from concourse.bass_utils import run_bass_kernel_spmd

F32 = mybir.dt.float32
F16 = mybir.dt.float16
AF = mybir.ActivationFunctionType

D = 1024          # model dim
S = 2048          # sequence length (per batch)
HPC = 4           # heads per core
DK = 64           # head dim
HC = HPC * DK     # head cols per core = 256
FC = 8            # feature chunks of 128 (contraction for projections)
TT = 4            # token tiles of 512
KC = 16           # key chunks of 128
LAG = 8           # ctx-matmul lag behind score-matmuls (steps)

LAST_RESULTS = None  # BassKernelResults of the most recent run (for test.py)
_NC_CACHE = None


# move_matmul_waits_to_ldweights emits a standalone InstLdweights per
# matmul, which walrus's LDW optimization refuses; skip it and let
# generate_event_semaphores legalize multi-waits via event semaphores.
bacc.Bacc.move_matmul_waits_to_ldweights = lambda self: None
_Bacc = bacc.Bacc


def build_nc():
    # Bacc (not raw Bass): its compile() runs generate_event_semaphores,
    # which legalizes multi-semaphore waits down to the hardware limit.
    nc = _Bacc("TRN2", target_bir_lowering=False, debug=False)

    xq = nc.dram_tensor("xq_t", [128, TT, FC, 512], F16, kind="ExternalInput")
    xk = nc.dram_tensor("xk_t", [128, TT, FC, 512], F16, kind="ExternalInput")
    xv = nc.dram_tensor("xv_t", [128, TT, FC, 512], F16, kind="ExternalInput")
    wq = nc.dram_tensor("wq", [128, FC, HC], F16, kind="ExternalInput")
    wk = nc.dram_tensor("wk", [128, FC, HC], F16, kind="ExternalInput")
    wv = nc.dram_tensor("wv", [128, FC, HC], F16, kind="ExternalInput")
    wo = nc.dram_tensor("wo", [128, 2, D], F16, kind="ExternalInput")
    bq = nc.dram_tensor("bq2", [128, 2], F32, kind="ExternalInput")
    bk = nc.dram_tensor("bk2", [128, 2], F32, kind="ExternalInput")
    bv = nc.dram_tensor("bv_bc", [128, HC], F32, kind="ExternalInput")
    out_p = nc.dram_tensor("out_p", [D, S], F16, kind="ExternalOutput")

    with tile.TileContext(nc) as tc:
        _emit(tc, xq, xk, xv, wq, wk, wv, wo, bq, bk, bv, out_p)
    nc.compile()
    return nc


def _emit(tc, xq, xk, xv, wq, wk, wv, wo, bq, bk, bv, out_p):
    nc = tc.nc

    with (
        nc.allow_low_precision(
            reason="fp16 matmul operands; all magnitudes well within fp16 range"
        ),
        tc.tile_pool(name="const", bufs=1) as cpool,
        tc.tile_pool(name="big", bufs=1) as bigpool,
        tc.tile_pool(name="xin", bufs=8) as xin,
        tc.tile_pool(name="pT", bufs=10) as ptpool,
        tc.tile_pool(name="rc", bufs=4) as rcpool,
        tc.tile_pool(name="osb", bufs=2) as osb,
        tc.tile_pool(name="ps", bufs=1, space="PSUM") as psum,
    ):
        # ---- resident weights / biases ----
        wq_sb = cpool.tile([128, FC, HC], F16, tag="wq")
        wk_sb = cpool.tile([128, FC, HC], F16, tag="wk")
        wv_sb = cpool.tile([128, FC, HC], F16, tag="wv")
        wo_sb = cpool.tile([128, 2, D], F16, tag="wo")
        bq_sb = cpool.tile([128, 2], F32, tag="bq")
        bk_sb = cpool.tile([128, 2], F32, tag="bk")
        bv_sb = cpool.tile([128, HC], F32, tag="bv")

        # ---- resident activations ----
        kT_sb = bigpool.tile([128, 2, S], F16, tag="kT")        # K^T (2 m-tiles)
        v_sb = bigpool.tile([128, HPC, KC, 128], F16, tag="v")  # V natural +1s+0pad
        qT_sb = [
            bigpool.tile([128, 2, 512], F16, tag=f"qT{t}", name=f"qT{t}")
            for t in range(TT)
        ]
        cT_sb = [
            bigpool.tile([128, 2, 512], F16, tag=f"cT{t}", name=f"cT{t}")
            for t in range(TT)
        ]

        # ---- loads: one engine queue = strict priority order; descriptors
        # fan out across the 16 hardware DMA queues for full bandwidth ----
        def load_x(x_dram, t):
            xt = xin.tile([128, FC, 512], F16, tag="xin", name=f"x{t}")
            nc.gpsimd.dma_start(xt[:], x_dram[:, t])
            return xt

        nc.gpsimd.dma_start(wk_sb[:], wk[:])
        nc.gpsimd.dma_start(wq_sb[:], wq[:])
        xk_t, xq_t, xv_t = [None] * TT, [None] * TT, [None] * TT
        xk_t[0] = load_x(xk, 0)
        xq_t[0] = load_x(xq, 0)
        xk_t[1] = load_x(xk, 1)
        nc.gpsimd.dma_start(wv_sb[:], wv[:])
        xv_t[0] = load_x(xv, 0)
        xk_t[2] = load_x(xk, 2)
        xv_t[1] = load_x(xv, 1)
        xk_t[3] = load_x(xk, 3)
        xv_t[2] = load_x(xv, 2)
        xv_t[3] = load_x(xv, 3)
        for t in range(1, TT):
            xq_t[t] = load_x(xq, t)
        nc.gpsimd.dma_start(wo_sb[:], wo[:])
        nc.sync.dma_start(bq_sb[:], bq[:])
        nc.sync.dma_start(bk_sb[:], bk[:])
        nc.sync.dma_start(bv_sb[:], bv[:])

        # ---- warmup: keep the PE activity monitor busy through the initial
        # DMA wait (else the projections run at the 1.2 GHz throttled clock)
        # and pull the ~2.7us exp table load off the first real activation ----
        warm_sb = cpool.tile([128, 512], F16, tag="warm")
        nc.vector.memset(warm_sb[:], 0.0)
        warm_ps = psum.tile([128, 512], F32, tag="O", bufs=2, name="warm_ps")
        for _ in range(40):
            nc.tensor.matmul(warm_ps[:], lhsT=warm_sb[:, 0:128],
                             rhs=warm_sb[:], start=True, stop=True)
        warm_act = cpool.tile([1, 16], F32, tag="warm_act")
        nc.scalar.activation(warm_act[:], warm_sb[0:1, 0:16], AF.Exp, scale=1.0)

        ones_f32 = cpool.tile([128, DK], F32, tag="ones_f32")
        nc.vector.memset(ones_f32[:], 1.0)
        for h in range(HPC):
            nc.vector.tensor_copy(
                v_sb[:, h, :, DK : DK + 1],
                ones_f32[:, 0:KC].rearrange("p (f o) -> p f o", o=1),
            )
            # zero the pad columns so the full-width ctx matmuls (M=128 keeps
            # the PE activity monitor warm + enables FWL) add only zeros
            nc.vector.memset(v_sb[:, h, :, DK + 1 : 128], 0.0)

        # ---- projection emitters ----
        def proj_T_mt(xt, w_sb, b_sb, dst, mt, tag):
            # dst -> AP [128, 512]; computes (x @ W)^T + b for one 128-col chunk
            ps = psum.tile([128, 512], F32, tag=tag, bufs=2,
                           name="psp")
            for f in range(FC):
                nc.tensor.matmul(
                    ps[:],
                    lhsT=w_sb[:, f, mt * 128 : (mt + 1) * 128],
                    rhs=xt[:, f, :],
                    start=(f == 0),
                    stop=(f == FC - 1),
                )
            nc.vector.tensor_scalar_add(dst, ps[:], b_sb[:, mt : mt + 1])

        def v_proj_j(t, j, tag):
            # V in natural layout [tokens, cols], with bias broadcast tensor
            kt = t * 4 + j
            ps = psum.tile([128, HC], F32, tag=tag, bufs=2,
                           name="vps")
            for f in range(FC):
                nc.tensor.matmul(
                    ps[:],
                    lhsT=xv_t[t][:, f, j * 128 : (j + 1) * 128],
                    rhs=wv_sb[:, f, :],
                    start=(f == 0),
                    stop=(f == FC - 1),
                )
            nc.vector.tensor_add(
                v_sb[:, :, kt, 0:DK],
                ps[:].rearrange("p (h c) -> p h c", h=HPC),
                bv_sb[:].rearrange("p (h c) -> p h c", h=HPC),
            )

        # ---- phase 1 head: K(t0) + Q0 only, so the first exp fires as soon
        # as the first two input tiles land; K(t1..3) and all of V ride the
        # fillers with emission deadlines ahead of their consumers ----
        for mt in range(2):
            proj_T_mt(xk_t[0], wk_sb, bk_sb, kT_sb[:, mt, 0:512], mt, "A")
        for mt in range(2):
            proj_T_mt(xq_t[0], wq_sb, bq_sb, qT_sb[0][:, mt, :], mt, "A")

        # ---- filler machinery ----
        # filler units are emitted between attention steps; they allocate PSUM
        # only from tag "O" so the score double-buffer rotation stays clean.
        fillers = []       # pumped 2 at a time at odd-kc steps
        boundary = []      # pumped at kc==0 steps (after the C handoff)

        def pump(queue, n):
            for _ in range(n):
                if queue:
                    queue.pop(0)()
                elif queue is boundary and fillers:
                    fillers.pop(0)()

        def o_proj_units(qt):
            # output projection for token tile qt, computed TRANSPOSED
            # ([out-feature, token]; the host untransposes): wo is the
            # stationary operand (resident since t=0) and the
            # freshly-written cT streams as the moving operand, whose read
            # is protected by the matmul's own semaphore wait.  Partial
            # (host sums head groups).  2 units per 128-outcol chunk.
            units = []
            for oc in range(8):
                box = {}

                def mms(oc=oc, box=box):
                    o_ps = psum.tile([128, 512], F32, tag="O", bufs=2, name="ops")
                    box["ps"] = o_ps
                    for c2 in range(2):
                        nc.tensor.matmul(
                            o_ps[:],
                            lhsT=wo_sb[:, c2, oc * 128 : (oc + 1) * 128],
                            rhs=cT_sb[qt][:, c2, :],
                            start=(c2 == 0),
                            stop=(c2 == 1),
                        )

                def store(oc=oc, box=box):
                    ob = osb.tile([128, 512], F16, tag="ob")
                    nc.vector.tensor_copy(ob[:], box["ps"][:])
                    # alternate store queues so back-to-back stores overlap
                    q = nc.sync if oc % 2 == 0 else nc.gpsimd
                    q.dma_start(
                        out_p[oc * 128 : (oc + 1) * 128,
                              qt * 512 : (qt + 1) * 512],
                        ob[:],
                    )

                units += [mms, store]
            return units

        def enqueue_fillers(qt):
            if qt == 0:
                # V(t) units; each is pumped (LAG-deep) ahead of av(kc=4t)
                for t in range(TT):
                    for j in range(4):
                        fillers.append(lambda t=t, j=j: v_proj_j(t, j, "O"))
            else:
                o_units = o_proj_units(qt - 1)
                fillers.extend(o_units[:12])   # oc0..oc5
                boundary.extend(o_units[12:])  # oc6, oc7 cover the boundaries
            if qt < TT - 1:
                for mt in range(2):
                    fillers.append(
                        lambda t=qt + 1, mt=mt: proj_T_mt(
                            xq_t[t], wq_sb, bq_sb, qT_sb[t][:, mt, :], mt, "O"
                        )
                    )

        # ---- attention: one global software pipeline over 128 steps ----
        steps = [(qt, hp, kc) for qt in range(TT) for hp in range(2)
                 for kc in range(KC)]
        Cs = {}
        Ps = {}

        def sc_exp(qt, hp, kc):
            A = psum.tile([128, 2, 512], F32, tag="A", bufs=2, name="A")
            for i in range(2):
                p0 = i * 64
                # the adjacent row-packed score matmuls (rows 0:64 / 64:128
                # via lhsT base_partition) run concurrently in disjoint PE
                # row groups
                nc.tensor.matmul(
                    A[:, i, :],
                    lhsT=kT_sb[p0 : p0 + 64, hp, kc * 128 : (kc + 1) * 128],
                    rhs=qT_sb[qt][p0 : p0 + 64, hp, :],
                    start=True,
                    stop=True,
                )
            P = ptpool.tile([128, 2, 512], F16, tag="pT", name="P")
            nc.scalar.activation(
                P[:].rearrange("p a b -> p (a b)"),
                A[:].rearrange("p a b -> p (a b)"),
                AF.Exp,
                scale=0.125,
            )
            Ps[(qt, hp, kc)] = P

        def av(qt, hp, kc):
            if kc == 0:
                Cs[(qt, hp)] = psum.tile([128, 2, 512], F32, tag="C", bufs=1,
                                         name="C")
            C = Cs[(qt, hp)]
            P = Ps.pop((qt, hp, kc))
            for i in range(2):
                nc.tensor.matmul(
                    C[:, i, :],
                    lhsT=v_sb[:, 2 * hp + i, kc, :],
                    rhs=P[:, i, :],
                    start=(kc == 0),
                    stop=(kc == KC - 1),
                )

        def normalize(qt, hp):
            C = Cs.pop((qt, hp))
            # copy ctx+denominator out of PSUM first: the C accumulator is
            # released after these two copies, so the next head-pair's ctx
            # matmuls only wait ~1.3us; the rest runs off the critical path.
            # For the very last head-pair nothing waits on C, so skip the
            # staging copies and shorten the end-of-kernel serial chain.
            last = qt == TT - 1 and hp == 1
            cUs = []
            for i in range(2):
                if last:
                    cUs.append(C[:, i, :])
                    continue
                cU = rcpool.tile([DK + 1, 512], F32, tag="cU", name="cU")
                nc.vector.tensor_copy(cU[:], C[0 : DK + 1, i, :])
                cUs.append(cU)
            for i, cU in enumerate(cUs):
                p0 = i * 64
                # custom-DVE ops ignore the input base partition, so the
                # denominator row must be relocated to p0 by a builtin copy
                drow = rcpool.tile([1, 512], F32, tag="drow")
                nc.vector.tensor_copy(drow[:], cU[DK : DK + 1, :])
                rc = rcpool.tile([1, 512], F32, tag="rc")
                nc.vector.reciprocal_approx_fast(rc[:], drow[:])
                Sb = rcpool.tile([DK, 512], F32, tag="Sb")
                nc.gpsimd.partition_broadcast(Sb[:], rc[:])
                nc.vector.tensor_mul(
                    cT_sb[qt][p0 : p0 + 64, hp, :], cU[0:DK, :], Sb[:]
                )

        for idx in range(len(steps) + LAG):
            if idx < len(steps):
                qt, hp, kc = steps[idx]
                # K(t) projection lands right before the first score matmul
                # that reads kT[t]; PE naturally waits for the xk(t) DMA here
                if qt == 0 and hp == 0 and kc in (4, 8, 12):
                    for mt in range(2):
                        proj_T_mt(
                            xk_t[kc // 4], wk_sb, bk_sb,
                            kT_sb[:, mt, kc * 128 : (kc + 4) * 128], mt, "O",
                        )
                sc_exp(qt, hp, kc)
            if idx >= LAG:
                qt, hp, kc = steps[idx - LAG]
                # enqueue on the av side: qt's units may read cT[qt-1], whose
                # last writer (normalize of qt-1/hp1) was emitted one av-step
                # earlier -- enqueueing on the sc side would let the tail of
                # qt-1's pump sites emit them too early
                if hp == 0 and kc == 0:
                    enqueue_fillers(qt)
                # qt0's K/V/Q units must be emitted BEFORE the av/sc steps
                # that consume them (pump-before-av); qt>0's O units read
                # cT[qt-1] whose writers land just before av(kc0), so the
                # boundary pump goes after it
                if qt == 0 or kc % 2 == 1:
                    pump(fillers, 2)
                av(qt, hp, kc)
                if kc == 0 and qt > 0:
                    pump(boundary, 2)
                if kc == KC - 1:
                    normalize(qt, hp)

        # tail: flush leftovers, then the last token tile's output projection
        pump(fillers, len(fillers))
        pump(boundary, len(boundary))
        for u in o_proj_units(TT - 1):
            u()


def _tile_x(xb):
    # [D, S] -> [128, TT, FC, 512] with X[p, t, f, s] = x[f*128 + p, t*512 + s]
    # so each 512-token tile is one fully contiguous 8KB-per-partition DMA
    return np.ascontiguousarray(
        xb.reshape(FC, 128, TT, 512).transpose(1, 2, 0, 3).astype(np.float16)
    )


def _tile_w(w):
    # [D, C] -> [128, FC, C] with W[p, f, c] = w[f*128 + p, c]
    c = w.shape[1]
    return np.ascontiguousarray(
        w.reshape(FC, 128, c).transpose(1, 0, 2).astype(np.float16)
    )


def _tile_wo(w):
    # [HC, D] -> [128, 2, D]
    return np.ascontiguousarray(
        w.reshape(2, 128, D).transpose(1, 0, 2).astype(np.float16)
    )


def _shard_inputs(query, key_in, value, Wq, bq, Wk, bk, Wv, bv, Wo, bo):
    q = np.asarray(query, dtype=np.float32)
    k = np.asarray(key_in, dtype=np.float32)
    v = np.asarray(value, dtype=np.float32)
    Wq, Wk, Wv, Wo = (np.asarray(a, np.float32) for a in (Wq, Wk, Wv, Wo))
    bq, bk, bv = (np.asarray(a, np.float32) for a in (bq, bk, bv))

    # per-batch tiled fp16 activations, shared across the 4 head groups
    xT = {b: tuple(_tile_x(x[b].T) for x in (q, k, v)) for b in range(2)}

    in_maps = []
    for core in range(8):
        b, g = divmod(core, 4)
        sl = slice(g * HC, (g + 1) * HC)
        xq_t, xk_t, xv_t = xT[b]
        in_maps.append(
            {
                "xq_t": xq_t,
                "xk_t": xk_t,
                "xv_t": xv_t,
                "wq": _tile_w(Wq[:, sl]),
                "wk": _tile_w(Wk[:, sl]),
                "wv": _tile_w(Wv[:, sl]),
                "wo": _tile_wo(Wo[sl, :]),
                "bq2": np.ascontiguousarray(bq[sl].reshape(2, 128).T),
                "bk2": np.ascontiguousarray(bk[sl].reshape(2, 128).T),
                "bv_bc": np.ascontiguousarray(
                    np.broadcast_to(bv[sl], (128, HC))
                ),
            }
        )
    return in_maps


def kernel(query=None, key_in=None, value=None, Wq=None, bq=None, Wk=None,
           bk=None, Wv=None, bv=None, Wo=None, bo=None, key=None, **_unused):
    global LAST_RESULTS, _NC_CACHE
    if key_in is None:
        key_in = key
    if _NC_CACHE is None:
        _NC_CACHE = build_nc()
    nc = _NC_CACHE

    in_maps = _shard_inputs(query, key_in, value, Wq, bq, Wk, bk, Wv, bv, Wo, bo)
    trace = bool(os.environ.get("BASS_TRACE"))
    res = run_bass_kernel_spmd(nc, in_maps, core_ids=list(range(8)), trace=trace)
    LAST_RESULTS = res

    bo = np.asarray(bo, np.float32)
    out = np.empty((2, S, D), dtype=np.float32)
    for b in range(2):
        acc = res.results[4 * b]["out_p"].astype(np.float32)
        for g in range(1, 4):
            acc = acc + res.results[4 * b + g]["out_p"].astype(np.float32)
        out[b] = acc.T + bo
    return out


# revision 33
# speedup vs baseline: 1.0272x; 1.0272x over previous
"""Trainium2 Bass kernel for 16-head MultiHeadAttention (B=2, S=2048, D=1024).

Sharding: 8 cores = 2 (batch) x 4 (head groups of 4 heads).  Each core
computes, for its batch b and head group g:
  Q_g = x_q @ Wq[:, g] ; K_g, V_g likewise
  ctx_g = softmax(Q_g K_g^T / sqrt(64)) V_g            (4 heads)
  out_partial = ctx_g @ Wo[g, :]                        [2048, 1024]
Host sums the 4 partials per batch and adds bo.

v3 layout/schedule notes:
  - inputs/weights are pre-cast to fp16 AND pre-tiled on the host so every
    DMA moves fully contiguous 8KB-per-partition lines at peak HBM rate;
    output partials are fp16
  - activations are fed transposed (features on partitions) so every matmul
    contracts over the partition dim without any on-device transposes
  - scores are computed transposed (s^T[keys, queries]) so the exp'd
    probabilities feed the ctx matmul directly; softmax skips
    max-subtraction (scores ~ N(0,1)); denominators come from a ones
    column appended to V
  - PSUM: tag A [128,2,512]x2 (double-buffered scores + phase-1
    projections), tag C [128,2,512]x1 (ctx accum, both heads), tag O
    [128,1024]x1 (out-proj + mid-attention filler projections) = 8 banks
  - one global software pipeline over all 128 (qt, hp, kc) steps: ctx
    matmuls lag the score matmuls by LAG steps, and out-proj / Q-proj /
    V-proj work is injected as "filler" units between steps so the
    in-order PE stream never blocks the exp cadence on ScalarE (the
    bottleneck engine)
  - softmax: ctx+denominator rows are copied out of PSUM first (frees the
    C accumulator after ~1.3us), then 1/den via reciprocal_approx_fast
    (~51 ULP) + gpsimd partition_broadcast + multiply, off the PE
    critical path.  Custom-DVE ops ignore the input base partition, so
    the denominator row is relocated to p0 by a builtin copy first.
"""

import os
import sys

sys.path.insert(0, "/opt/trn_rl_repo")

import numpy as np

import concourse.bass as bass
import concourse.tile as tile
from concourse import bacc, mybir
from concourse.bass_utils import run_bass_kernel_spmd

F32 = mybir.dt.float32
F16 = mybir.dt.float16
AF = mybir.ActivationFunctionType

D = 1024          # model dim
S = 2048          # sequence length (per batch)
HPC = 4           # heads per core
DK = 64           # head dim
HC = HPC * DK     # head cols per core = 256
FC = 8            # feature chunks of 128 (contraction for projections)
TT = 4            # token tiles of 512
KC = 16           # key chunks of 128
LAG = 16          # ctx-matmul lag behind score-matmuls (steps)

LAST_RESULTS = None  # BassKernelResults of the most recent run (for test.py)
_NC_CACHE = None


# move_matmul_waits_to_ldweights emits a standalone InstLdweights per
# matmul, which walrus's LDW optimization refuses; skip it and let
# generate_event_semaphores legalize multi-waits via event semaphores.
bacc.Bacc.move_matmul_waits_to_ldweights = lambda self: None
_Bacc = bacc.Bacc


def build_nc():
    # Bacc (not raw Bass): its compile() runs generate_event_semaphores,
    # which legalizes multi-semaphore waits down to the hardware limit.
    nc = _Bacc("TRN2", target_bir_lowering=False, debug=False)

    xq = nc.dram_tensor("xq_t", [128, TT, FC, 512], F16, kind="ExternalInput")
    xk = nc.dram_tensor("xk_t", [128, TT, FC, 512], F16, kind="ExternalInput")
    xv = nc.dram_tensor("xv_t", [128, TT, FC, 512], F16, kind="ExternalInput")
    wq = nc.dram_tensor("wq", [128, FC, HC], F16, kind="ExternalInput")
    wk = nc.dram_tensor("wk", [128, FC, HC], F16, kind="ExternalInput")
    wv = nc.dram_tensor("wv", [128, FC, HC], F16, kind="ExternalInput")
    wo = nc.dram_tensor("wo", [128, 2, D], F16, kind="ExternalInput")
    bq = nc.dram_tensor("bq2", [128, 2], F32, kind="ExternalInput")
    bk = nc.dram_tensor("bk2", [128, 2], F32, kind="ExternalInput")
    bv = nc.dram_tensor("bv_bc", [128, HC], F32, kind="ExternalInput")
    out_p = nc.dram_tensor("out_p", [D, S], F16, kind="ExternalOutput")

    with tile.TileContext(nc) as tc:
        _emit(tc, xq, xk, xv, wq, wk, wv, wo, bq, bk, bv, out_p)
    nc.compile()
    return nc


def _emit(tc, xq, xk, xv, wq, wk, wv, wo, bq, bk, bv, out_p):
    nc = tc.nc

    with (
        nc.allow_low_precision(
            reason="fp16 matmul operands; all magnitudes well within fp16 range"
        ),
        tc.tile_pool(name="const", bufs=1) as cpool,
        tc.tile_pool(name="big", bufs=1) as bigpool,
        tc.tile_pool(name="xin", bufs=8) as xin,
        tc.tile_pool(name="pT", bufs=18) as ptpool,
        tc.tile_pool(name="rc", bufs=4) as rcpool,
        tc.tile_pool(name="osb", bufs=2) as osb,
        tc.tile_pool(name="ps", bufs=1, space="PSUM") as psum,
    ):
        # ---- resident weights / biases ----
        wq_sb = cpool.tile([128, FC, HC], F16, tag="wq")
        wk_sb = cpool.tile([128, FC, HC], F16, tag="wk")
        wv_sb = cpool.tile([128, FC, HC], F16, tag="wv")
        wo_sb = cpool.tile([128, 2, D], F16, tag="wo")
        bq_sb = cpool.tile([128, 2], F32, tag="bq")
        bk_sb = cpool.tile([128, 2], F32, tag="bk")
        bv_sb = cpool.tile([128, HC], F32, tag="bv")

        # ---- resident activations ----
        kT_sb = bigpool.tile([128, 2, S], F16, tag="kT")        # K^T (2 m-tiles)
        v_sb = bigpool.tile([128, HPC, KC, 128], F16, tag="v")  # V natural +1s+0pad
        qT_sb = [
            bigpool.tile([128, 2, 512], F16, tag=f"qT{t}", name=f"qT{t}")
            for t in range(TT)
        ]
        cT_sb = [
            bigpool.tile([128, 2, 512], F16, tag=f"cT{t}", name=f"cT{t}")
            for t in range(TT)
        ]

        # ---- loads: one engine queue = strict priority order; descriptors
        # fan out across the 16 hardware DMA queues for full bandwidth ----
        def load_x(x_dram, t):
            xt = xin.tile([128, FC, 512], F16, tag="xin", name=f"x{t}")
            nc.gpsimd.dma_start(xt[:], x_dram[:, t])
            return xt

        nc.gpsimd.dma_start(wk_sb[:], wk[:])
        nc.gpsimd.dma_start(wq_sb[:], wq[:])
        xk_t, xq_t, xv_t = [None] * TT, [None] * TT, [None] * TT
        xk_t[0] = load_x(xk, 0)
        xq_t[0] = load_x(xq, 0)
        xk_t[1] = load_x(xk, 1)
        nc.gpsimd.dma_start(wv_sb[:], wv[:])
        xv_t[0] = load_x(xv, 0)
        xk_t[2] = load_x(xk, 2)
        xv_t[1] = load_x(xv, 1)
        xk_t[3] = load_x(xk, 3)
        xv_t[2] = load_x(xv, 2)
        xv_t[3] = load_x(xv, 3)
        for t in range(1, TT):
            xq_t[t] = load_x(xq, t)
        nc.gpsimd.dma_start(wo_sb[:], wo[:])
        nc.sync.dma_start(bq_sb[:], bq[:])
        nc.sync.dma_start(bk_sb[:], bk[:])
        nc.sync.dma_start(bv_sb[:], bv[:])

        # ---- warmup: keep the PE activity monitor busy through the initial
        # DMA wait (else the projections run at the 1.2 GHz throttled clock)
        # and pull the ~2.7us exp table load off the first real activation ----
        warm_sb = cpool.tile([128, 512], F16, tag="warm")
        nc.vector.memset(warm_sb[:], 0.0)
        warm_ps = psum.tile([128, 512], F32, tag="O", bufs=2, name="warm_ps")
        for _ in range(32):
            nc.tensor.matmul(warm_ps[:], lhsT=warm_sb[:, 0:128],
                             rhs=warm_sb[:], start=True, stop=True)
        warm_act = cpool.tile([1, 16], F32, tag="warm_act")
        nc.scalar.activation(warm_act[:], warm_sb[0:1, 0:16], AF.Exp, scale=1.0)

        ones_f32 = cpool.tile([128, DK], F32, tag="ones_f32")
        nc.vector.memset(ones_f32[:], 1.0)
        for h in range(HPC):
            nc.vector.tensor_copy(
                v_sb[:, h, :, DK : DK + 1],
                ones_f32[:, 0:KC].rearrange("p (f o) -> p f o", o=1),
            )
            # zero the pad columns so the full-width ctx matmuls (M=128 keeps
            # the PE activity monitor warm + enables FWL) add only zeros
            nc.vector.memset(v_sb[:, h, :, DK + 1 : 128], 0.0)

        # ---- projection emitters ----
        def proj_T_mt(xt, w_sb, b_sb, dst, mt, tag):
            # dst -> AP [128, 512]; computes (x @ W)^T + b for one 128-col chunk
            ps = psum.tile([128, 512], F32, tag=tag, bufs=2,
                           name="psp")
            for f in range(FC):
                nc.tensor.matmul(
                    ps[:],
                    lhsT=w_sb[:, f, mt * 128 : (mt + 1) * 128],
                    rhs=xt[:, f, :],
                    start=(f == 0),
                    stop=(f == FC - 1),
                )
            nc.vector.tensor_scalar_add(dst, ps[:], b_sb[:, mt : mt + 1])

        def v_proj_j(t, j, tag):
            # V in natural layout [tokens, cols], with bias broadcast tensor
            kt = t * 4 + j
            ps = psum.tile([128, HC], F32, tag=tag, bufs=2,
                           name="vps")
            for f in range(FC):
                nc.tensor.matmul(
                    ps[:],
                    lhsT=xv_t[t][:, f, j * 128 : (j + 1) * 128],
                    rhs=wv_sb[:, f, :],
                    start=(f == 0),
                    stop=(f == FC - 1),
                )
            nc.vector.tensor_add(
                v_sb[:, :, kt, 0:DK],
                ps[:].rearrange("p (h c) -> p h c", h=HPC),
                bv_sb[:].rearrange("p (h c) -> p h c", h=HPC),
            )

        # ---- phase 1 head: K(t0) + Q0 only, so the first exp fires as soon
        # as the first two input tiles land; K(t1..3) and all of V ride the
        # fillers with emission deadlines ahead of their consumers ----
        for mt in range(2):
            proj_T_mt(xk_t[0], wk_sb, bk_sb, kT_sb[:, mt, 0:512], mt, "A")
        for mt in range(2):
            proj_T_mt(xq_t[0], wq_sb, bq_sb, qT_sb[0][:, mt, :], mt, "A")

        # ---- filler machinery ----
        # filler units are emitted between attention steps; they allocate PSUM
        # only from tag "O" so the score double-buffer rotation stays clean.
        fillers = []       # pumped 2 at a time at odd-kc steps
        boundary = []      # pumped at kc==0 steps (after the C handoff)

        def pump(queue, n):
            for _ in range(n):
                if queue:
                    queue.pop(0)()
                elif queue is boundary and fillers:
                    fillers.pop(0)()

        def o_proj_units(qt):
            # output projection for token tile qt, computed TRANSPOSED
            # ([out-feature, token]; the host untransposes): wo is the
            # stationary operand (resident since t=0) and the
            # freshly-written cT streams as the moving operand, whose read
            # is protected by the matmul's own semaphore wait.  Partial
            # (host sums head groups).  2 units per 128-outcol chunk.
            units = []
            for oc in range(8):
                box = {}

                def mms(oc=oc, box=box):
                    o_ps = psum.tile([128, 512], F32, tag="O", bufs=2, name="ops")
                    box["ps"] = o_ps
                    for c2 in range(2):
                        nc.tensor.matmul(
                            o_ps[:],
                            lhsT=wo_sb[:, c2, oc * 128 : (oc + 1) * 128],
                            rhs=cT_sb[qt][:, c2, :],
                            start=(c2 == 0),
                            stop=(c2 == 1),
                        )

                def store(oc=oc, box=box):
                    ob = osb.tile([128, 512], F16, tag="ob")
                    nc.vector.tensor_copy(ob[:], box["ps"][:])
                    # alternate store queues so back-to-back stores overlap
                    q = nc.sync if oc % 2 == 0 else nc.gpsimd
                    q.dma_start(
                        out_p[oc * 128 : (oc + 1) * 128,
                              qt * 512 : (qt + 1) * 512],
                        ob[:],
                    )

                units += [mms, store]
            return units

        def enqueue_fillers(qt):
            if qt == 0:
                # V(t2)/V(t3) units pumped just-in-time ahead of av(kc=4t);
                # V(t0)/V(t1) are emitted inline on the score side where the
                # PE is otherwise idle while ScalarE paces
                for t in range(2, TT):
                    for j in range(4):
                        fillers.append(lambda t=t, j=j: v_proj_j(t, j, "O"))
            if qt < TT - 1:
                # Q units first: they carry no cT dependency, so the first
                # pump sites after a qt boundary never stall on the previous
                # tile's normalize chain
                for mt in range(2):
                    fillers.append(
                        lambda t=qt + 1, mt=mt: proj_T_mt(
                            xq_t[t], wq_sb, bq_sb, qT_sb[t][:, mt, :], mt, "O"
                        )
                    )
            if qt > 0:
                fillers.extend(o_proj_units(qt - 1))

        # ---- attention: one global software pipeline over 128 steps ----
        steps = [(qt, hp, kc) for qt in range(TT) for hp in range(2)
                 for kc in range(KC)]
        Cs = {}
        Ps = {}

        def sc_exp(qt, hp, kc):
            A = psum.tile([128, 2, 512], F32, tag="A", bufs=2, name="A")
            for i in range(2):
                p0 = i * 64
                # the adjacent row-packed score matmuls (rows 0:64 / 64:128
                # via lhsT base_partition) run concurrently in disjoint PE
                # row groups
                nc.tensor.matmul(
                    A[:, i, :],
                    lhsT=kT_sb[p0 : p0 + 64, hp, kc * 128 : (kc + 1) * 128],
                    rhs=qT_sb[qt][p0 : p0 + 64, hp, :],
                    start=True,
                    stop=True,
                )
            P = ptpool.tile([128, 2, 512], F16, tag="pT", name="P")
            nc.scalar.activation(
                P[:].rearrange("p a b -> p (a b)"),
                A[:].rearrange("p a b -> p (a b)"),
                AF.Exp,
                scale=0.125,
            )
            Ps[(qt, hp, kc)] = P

        def av(qt, hp, kc):
            if kc == 0:
                Cs[(qt, hp)] = psum.tile([128, 2, 512], F32, tag="C", bufs=1,
                                         name="C")
            C = Cs[(qt, hp)]
            P = Ps.pop((qt, hp, kc))
            for i in range(2):
                nc.tensor.matmul(
                    C[:, i, :],
                    lhsT=v_sb[:, 2 * hp + i, kc, :],
                    rhs=P[:, i, :],
                    start=(kc == 0),
                    stop=(kc == KC - 1),
                )

        def normalize(qt, hp):
            C = Cs.pop((qt, hp))
            # copy ctx+denominator out of PSUM first: the C accumulator is
            # released after these two copies, so the next head-pair's ctx
            # matmuls only wait ~1.3us; the rest runs off the critical path.
            # For the very last head-pair nothing waits on C, so skip the
            # staging copies and shorten the end-of-kernel serial chain.
            last = qt == TT - 1 and hp == 1
            cUs = []
            for i in range(2):
                if last:
                    cUs.append(C[:, i, :])
                    continue
                cU = rcpool.tile([DK + 1, 512], F32, tag="cU", name="cU")
                nc.vector.tensor_copy(cU[:], C[0 : DK + 1, i, :])
                cUs.append(cU)
            for i, cU in enumerate(cUs):
                p0 = i * 64
                # custom-DVE ops ignore the input base partition, so the
                # denominator row must be relocated to p0 by a builtin copy
                drow = rcpool.tile([1, 512], F32, tag="drow")
                nc.vector.tensor_copy(drow[:], cU[DK : DK + 1, :])
                rc = rcpool.tile([1, 512], F32, tag="rc")
                nc.vector.reciprocal_approx_fast(rc[:], drow[:])
                Sb = rcpool.tile([DK, 512], F32, tag="Sb")
                nc.gpsimd.partition_broadcast(Sb[:], rc[:])
                nc.vector.tensor_mul(
                    cT_sb[qt][p0 : p0 + 64, hp, :], cU[0:DK, :], Sb[:]
                )

        for idx in range(len(steps) + LAG):
            if idx < len(steps):
                qt, hp, kc = steps[idx]
                # K(t) projection lands right before the first score matmul
                # that reads kT[t]; PE naturally waits for the xk(t) DMA here.
                # V(t0)/V(t1) slot into the remaining early score-side steps
                # (one ~0.85us unit per ~1.03us exp step keeps ScalarE fed)
                if qt == 0 and hp == 0:
                    if kc in (4, 8, 12):
                        for mt in range(2):
                            proj_T_mt(
                                xk_t[kc // 4], wk_sb, bk_sb,
                                kT_sb[:, mt, kc * 128 : (kc + 4) * 128], mt, "O",
                            )
                    inline_v = {2: 0, 3: 1, 5: 2, 6: 3, 7: 4, 9: 5, 10: 6, 11: 7}
                    if kc in inline_v:
                        u = inline_v[kc]
                        v_proj_j(u // 4, u % 4, "O")
                sc_exp(qt, hp, kc)
            if idx >= LAG:
                qt, hp, kc = steps[idx - LAG]
                # enqueue on the av side: qt's units may read cT[qt-1], whose
                # last writer (normalize of qt-1/hp1) was emitted one av-step
                # earlier -- enqueueing on the sc side would let the tail of
                # qt-1's pump sites emit them too early
                if hp == 0 and kc == 0:
                    enqueue_fillers(qt)
                # one filler unit per av site (~0.85us PE against ~1.03us of
                # exp): V/Q units must land BEFORE the avs that consume them;
                # at kc==0 the av must go first (the pumped O unit waits on
                # cT writers that land just before, and would delay the
                # C-accumulator handoff)
                if kc == 0:
                    av(qt, hp, kc)
                    pump(fillers, 1)
                else:
                    pump(fillers, 1)
                    av(qt, hp, kc)
                if kc == KC - 1:
                    normalize(qt, hp)

        # tail: flush leftovers, then the last token tile's output projection
        pump(fillers, len(fillers))
        pump(boundary, len(boundary))
        for u in o_proj_units(TT - 1):
            u()


def _tile_x(xb):
    # [D, S] -> [128, TT, FC, 512] with X[p, t, f, s] = x[f*128 + p, t*512 + s]
    # so each 512-token tile is one fully contiguous 8KB-per-partition DMA
    return np.ascontiguousarray(
        xb.reshape(FC, 128, TT, 512).transpose(1, 2, 0, 3).astype(np.float16)
    )


def _tile_w(w):
    # [D, C] -> [128, FC, C] with W[p, f, c] = w[f*128 + p, c]
    c = w.shape[1]
    return np.ascontiguousarray(
        w.reshape(FC, 128, c).transpose(1, 0, 2).astype(np.float16)
    )


def _tile_wo(w):
    # [HC, D] -> [128, 2, D]
    return np.ascontiguousarray(
        w.reshape(2, 128, D).transpose(1, 0, 2).astype(np.float16)
    )


def _shard_inputs(query, key_in, value, Wq, bq, Wk, bk, Wv, bv, Wo, bo):
    q = np.asarray(query, dtype=np.float32)
    k = np.asarray(key_in, dtype=np.float32)
    v = np.asarray(value, dtype=np.float32)
    Wq, Wk, Wv, Wo = (np.asarray(a, np.float32) for a in (Wq, Wk, Wv, Wo))
    bq, bk, bv = (np.asarray(a, np.float32) for a in (bq, bk, bv))

    # per-batch tiled fp16 activations, shared across the 4 head groups
    xT = {b: tuple(_tile_x(x[b].T) for x in (q, k, v)) for b in range(2)}

    in_maps = []
    for core in range(8):
        b, g = divmod(core, 4)
        sl = slice(g * HC, (g + 1) * HC)
        xq_t, xk_t, xv_t = xT[b]
        in_maps.append(
            {
                "xq_t": xq_t,
                "xk_t": xk_t,
                "xv_t": xv_t,
                "wq": _tile_w(Wq[:, sl]),
                "wk": _tile_w(Wk[:, sl]),
                "wv": _tile_w(Wv[:, sl]),
                "wo": _tile_wo(Wo[sl, :]),
                "bq2": np.ascontiguousarray(bq[sl].reshape(2, 128).T),
                "bk2": np.ascontiguousarray(bk[sl].reshape(2, 128).T),
                "bv_bc": np.ascontiguousarray(
                    np.broadcast_to(bv[sl], (128, HC))
                ),
            }
        )
    return in_maps


def kernel(query=None, key_in=None, value=None, Wq=None, bq=None, Wk=None,
           bk=None, Wv=None, bv=None, Wo=None, bo=None, key=None, **_unused):
    global LAST_RESULTS, _NC_CACHE
    if key_in is None:
        key_in = key
    if _NC_CACHE is None:
        _NC_CACHE = build_nc()
    nc = _NC_CACHE

    in_maps = _shard_inputs(query, key_in, value, Wq, bq, Wk, bk, Wv, bv, Wo, bo)
    trace = bool(os.environ.get("BASS_TRACE"))
    res = run_bass_kernel_spmd(nc, in_maps, core_ids=list(range(8)), trace=trace)
    LAST_RESULTS = res

    bo = np.asarray(bo, np.float32)
    out = np.empty((2, S, D), dtype=np.float32)
    for b in range(2):
        acc = res.results[4 * b]["out_p"].astype(np.float32)
        for g in range(1, 4):
            acc = acc + res.results[4 * b + g]["out_p"].astype(np.float32)
        out[b] = acc.T + bo
    return out
